# revision 17
# baseline (speedup 1.0000x reference)
"""Trainium2 Bass kernel for CorrCosine cost volumes.

Inputs (full): ref_features, cur_features [8, 256, 64, 64] f32.
out[b, hc, wc, hr, wr] = <cur_n[b, :, hc, wc], ref_n[b, :, hr, wr]>
where *_n are channel-L2-normalized features.

Sharding: data-parallel over batch B=8 across the 8 NeuronCores — each core
computes one batch's full [4096, 4096] cosine-similarity matrix:
  sim = (cur / |cur|_C).T @ (ref / |ref|_C)   with K = C = 256.

Per-core structure (Tile framework), v3:
  - both inputs stream in 512-pixel slices [128, 2, 512] as SWDGE cast-DMAs
    (f32 -> float32r rounds in flight; f32r = 8e11m fp32 that runs the PE at
    full speed for moving dim >= 256, ~2^-12 rounding). The SP HWDGE queue
    carries only the 64 MB of output stores.
  - ref slices: square (ACT) -> sumsq over C via ones-matmul -> sqrt (ACT)
    -> reciprocal (DVE) -> partition-broadcast via K=1 ones-matmul ->
    multiplied IN PLACE (reading the broadcast straight from PSUM).
  - cur stays unnormalized; inverse norms are reduced into output-row layout
    [128, 32] via N=1 matmuls and folded into the mandatory PSUM->SBUF
    copies (ACT activation scale / DVE tensor_scalar) for free.
  - main loop: 32 m-chunks x 8 n-tiles x 2 K-chunks of float32r matmuls,
    fused scaled copies alternating ScalarE/VectorE, 6 MB output DMAs
    (3 m-chunks per dma_start) on the SP queue.
"""

import numpy as np

import concourse.bass as bass
import concourse.mybir as mybir
import concourse.tile as tile
from concourse import bacc, bass_utils

B, C, H, W = 8, 256, 64, 64
HW = H * W           # 4096 pixels
KP = 128             # partitions per K-chunk
NK = C // KP         # 2 K-chunks
MT = 128             # output partition tile (cur pixels)
NT = 512             # output free tile (ref pixels) = one f32 PSUM bank
NM = HW // MT        # 32 m-chunks
NN = HW // NT        # 8 n-tiles
MO = 3               # m-chunks per output DMA (6 MB per dma_start)
MPS = NT // MT       # m-chunks per 512-pixel input slice = 4

F32 = mybir.dt.float32
F32R = mybir.dt.float32r
ACTF = mybir.ActivationFunctionType


def _kernel_body(tc, cur, ref, sim, repeats=1):
    nc = tc.nc
    with (
        tc.tile_pool(name="pers", bufs=1) as pers,
        tc.tile_pool(name="sqp", bufs=2) as sqp,
        tc.tile_pool(name="rowp", bufs=2) as rowp,
        tc.tile_pool(name="outp", bufs=2) as outp,
        tc.tile_pool(name="psmm", bufs=3, space=bass.MemorySpace.PSUM) as psmm,
        tc.tile_pool(name="pspre", bufs=2, space=bass.MemorySpace.PSUM) as pspre,
        tc.tile_pool(name="pscol", bufs=1, space=bass.MemorySpace.PSUM) as pscol,
    ):
        ones_col = pers.tile([KP, 1], F32, tag="ones_col")
        nc.vector.memset(ones_col, 1.0)
        ones_row = pers.tile([1, KP], F32, tag="ones_row")
        nc.vector.memset(ones_row, 1.0)

        cur_fr = pers.tile([KP, NK, HW], F32R, tag="cur_fr")
        ref_n = [
            pers.tile([KP, NK, NT], F32R, tag=f"ref_n{n}", name=f"ref_n{n}")
            for n in range(NN)
        ]
        inv_cur = pers.tile([KP, NM], F32, tag="inv_cur")

        cur_r = cur.rearrange("(k p) n -> p k n", p=KP)
        ref_r = ref.rearrange("(k p) n -> p k n", p=KP)

        sim_pm = sim.rearrange("(mm p) n -> p mm n", p=KP)
        for _rep in range(repeats):
            # cast-DMAs: all of ref first (every output tile reads all of
            # ref), cur slice 0 early (first m-chunks + inv_cur), rest of cur
            for n in range(NN):
                sl = slice(n * NT, (n + 1) * NT)
                nc.gpsimd.dma_start(out=ref_n[n], in_=ref_r[:, :, sl])
                if n == 0:
                    nc.gpsimd.dma_start(out=cur_fr[:, :, sl], in_=cur_r[:, :, sl])
            for n in range(1, NN):
                sl = slice(n * NT, (n + 1) * NT)
                nc.gpsimd.dma_start(out=cur_fr[:, :, sl], in_=cur_r[:, :, sl])

            def ref_chain(n):
                sq = sqp.tile([KP, NK, NT], F32, tag="sq_r", name="sq_r")
                nc.scalar.activation(sq, ref_n[n], ACTF.Square)
                pr = pspre.tile([1, NT], F32, tag="pre_row", name="pr")
                for k in range(NK):
                    nc.tensor.matmul(
                        pr, ones_col, sq[:, k, :], start=(k == 0), stop=(k == NK - 1)
                    )
                nrow = rowp.tile([1, NT], F32, tag="nrow", name="nrow")
                nc.scalar.activation(nrow, pr, ACTF.Sqrt)
                inv = rowp.tile([1, NT], F32, tag="inv", name="inv")
                nc.vector.reciprocal(inv, nrow)
                pb = pspre.tile([KP, NT], F32, tag="pre_bc", name="pb")
                nc.tensor.matmul(pb, ones_row, inv, start=True, stop=True)
                for k in range(NK):
                    nc.vector.tensor_mul(ref_n[n][:, k, :], ref_n[n][:, k, :], pb)

            def cur_chain(n):
                sl = slice(n * NT, (n + 1) * NT)
                csq = sqp.tile([KP, NK, NT], F32, tag="sq_c", name="sq_c")
                if n % 2 == 0:
                    nc.vector.tensor_mul(csq, cur_fr[:, :, sl], cur_fr[:, :, sl])
                else:
                    nc.scalar.activation(csq, cur_fr[:, :, sl], ACTF.Square)
                pcol = pscol.tile([KP, MPS], F32, tag="pre_col", name="pcol")
                for j in range(MPS):
                    for k in range(NK):
                        nc.tensor.matmul(
                            pcol[:, j:j + 1],
                            csq[:, k, j * MT:(j + 1) * MT],
                            ones_col,
                            start=(k == 0),
                            stop=(k == NK - 1),
                        )
                ncur = rowp.tile([KP, MPS], F32, tag="ncur", name="ncur")
                nc.scalar.activation(ncur, pcol, ACTF.Sqrt)
                nc.vector.reciprocal(inv_cur[:, n * MPS:(n + 1) * MPS], ncur)

            def out_group(mo, msz):
                out_sb = outp.tile([KP, MO, HW], F32, tag="out", name="out_sb")
                for mi in range(msz):
                    m = mo + mi
                    for n in range(NN):
                        ps = psmm.tile([KP, NT], F32, tag="mm", name="ps")
                        for k in range(NK):
                            nc.tensor.matmul(
                                ps,
                                cur_fr[:, k, m * MT:(m + 1) * MT],
                                ref_n[n][:, k, :],
                                start=(k == 0),
                                stop=(k == NK - 1),
                            )
                        dst = out_sb[:, mi, n * NT:(n + 1) * NT]
                        if n % 2 == 0:
                            nc.scalar.mul(dst, ps, inv_cur[:, m:m + 1])
                        else:
                            nc.vector.tensor_scalar_mul(dst, ps, inv_cur[:, m:m + 1])
                nc.sync.dma_start(
                    out=sim_pm[:, mo:mo + msz, :], in_=out_sb[:, :msz, :]
                )

            # ref chains first (every output column needs all of ref);
            # cur chains 0-1 cover the first 8 m-chunks, the rest are
            # emitted after the first output groups so the first 6 MB of
            # output copies don't queue behind them on DVE/ACT.
            for n in range(NN):
                ref_chain(n)
                if n < 2:
                    cur_chain(n)

            groups = [1, 2, 3, 3] + [3] * 7 + [2]
            mo = 0
            for gi, msz in enumerate(groups):
                out_group(mo, msz)
                mo += msz
                if gi == 2:
                    for n in range(2, NN):
                        cur_chain(n)
            assert mo == NM


_NC_CACHE = {}


def _get_nc(repeats=1):
    key = ("nc", repeats)
    if key not in _NC_CACHE:
        nc = bacc.Bacc("TRN2", target_bir_lowering=False, debug=False)
        cur_d = nc.dram_tensor("cur", [C, HW], F32, kind="ExternalInput")
        ref_d = nc.dram_tensor("ref", [C, HW], F32, kind="ExternalInput")
        sim_d = nc.dram_tensor("sim", [HW, HW], F32, kind="ExternalOutput")
        with tile.TileContext(nc) as tc:
            _kernel_body(tc, cur_d.ap(), ref_d.ap(), sim_d.ap(), repeats=repeats)
        nc.compile()
        _NC_CACHE[key] = nc
    return _NC_CACHE[key]


def kernel(ref_features, cur_features, _run_kwargs=None):
    ref_np = np.ascontiguousarray(
        np.asarray(ref_features, dtype=np.float32).reshape(B, C, HW)
    )
    cur_np = np.ascontiguousarray(
        np.asarray(cur_features, dtype=np.float32).reshape(B, C, HW)
    )
    nc = _get_nc()
    in_maps = [{"cur": cur_np[b], "ref": ref_np[b]} for b in range(B)]
    res = bass_utils.run_bass_kernel_spmd(
        nc, in_maps, core_ids=list(range(B)), **(_run_kwargs or {})
    )
    out = np.stack([res.results[b]["sim"] for b in range(B)], axis=0)
    if _run_kwargs is not None:
        _NC_CACHE["last_results"] = res
    return out.reshape(B, H, W, H, W)


# revision 18
# speedup vs baseline: 1.1262x; 1.1262x over previous
"""Trainium2 Bass kernel for CorrCosine cost volumes.

Inputs (full): ref_features, cur_features [8, 256, 64, 64] f32.
out[b, hc, wc, hr, wr] = <cur_n[b, :, hc, wc], ref_n[b, :, hr, wr]>
where *_n are channel-L2-normalized features.

Sharding: data-parallel over batch B=8 across the 8 NeuronCores — each core
computes one batch's full [4096, 4096] cosine-similarity matrix:
  sim = (cur / |cur|_C).T @ (ref / |ref|_C)   with K = C = 256.

Per-core structure (Tile framework), v3:
  - both inputs stream in 512-pixel slices [128, 2, 512] as SWDGE cast-DMAs
    (f32 -> float32r rounds in flight; f32r = 8e11m fp32 that runs the PE at
    full speed for moving dim >= 256, ~2^-12 rounding). The SP HWDGE queue
    carries only the 64 MB of output stores.
  - ref slices: square (ACT) -> sumsq over C via ones-matmul -> sqrt (ACT)
    -> reciprocal (DVE) -> partition-broadcast via K=1 ones-matmul ->
    multiplied IN PLACE (reading the broadcast straight from PSUM).
  - cur stays unnormalized; inverse norms are reduced into output-row layout
    [128, 32] via N=1 matmuls and folded into the mandatory PSUM->SBUF
    copies (ACT activation scale / DVE tensor_scalar) for free.
  - main loop: 32 m-chunks x 8 n-tiles x 2 K-chunks of float32r matmuls,
    fused scaled copies alternating ScalarE/VectorE, 6 MB output DMAs
    (3 m-chunks per dma_start) on the SP queue.
"""

import numpy as np

import concourse.bass as bass
import concourse.mybir as mybir
import concourse.tile as tile
from concourse import bacc, bass_utils

B, C, H, W = 8, 256, 64, 64
HW = H * W           # 4096 pixels
KP = 128             # partitions per K-chunk
NK = C // KP         # 2 K-chunks
MT = 128             # output partition tile (cur pixels)
NT = 512             # output free tile (ref pixels) = one f32 PSUM bank
NM = HW // MT        # 32 m-chunks
NN = HW // NT        # 8 n-tiles
MO = 3               # m-chunks per output DMA (6 MB per dma_start)
MPS = NT // MT       # m-chunks per 512-pixel input slice = 4

F32 = mybir.dt.float32
F32R = mybir.dt.float32r
ACTF = mybir.ActivationFunctionType


def _kernel_body(tc, cur, ref, sim, repeats=1):
    nc = tc.nc
    with (
        tc.tile_pool(name="pers", bufs=1) as pers,
        tc.tile_pool(name="sqp", bufs=2) as sqp,
        tc.tile_pool(name="rowp", bufs=2) as rowp,
        tc.tile_pool(name="outp", bufs=2) as outp,
        tc.tile_pool(name="psmm", bufs=5, space=bass.MemorySpace.PSUM) as psmm,
        tc.tile_pool(name="pspre", bufs=2, space=bass.MemorySpace.PSUM) as pspre,
        tc.tile_pool(name="pscol", bufs=1, space=bass.MemorySpace.PSUM) as pscol,
    ):
        ones_col = pers.tile([KP, 1], F32, tag="ones_col")
        nc.vector.memset(ones_col, 1.0)
        # all-ones [128, 128] f32r stationary operand: ones_mat.T @ sq gives
        # the per-column sums replicated across all 128 partitions, fusing
        # the partition-reduce and the broadcast into one full-speed matmul
        ones_f32 = pers.tile([KP, KP], F32, tag="ones_f32")
        nc.vector.memset(ones_f32, 1.0)
        ones_mat = pers.tile([KP, KP], F32R, tag="ones_mat")
        nc.scalar.copy(ones_mat, ones_f32)

        cur_fr = pers.tile([KP, NK, HW], F32R, tag="cur_fr")
        ref_n = [
            pers.tile([KP, NK, NT], F32R, tag=f"ref_n{n}", name=f"ref_n{n}")
            for n in range(NN)
        ]
        inv_cur = pers.tile([KP, NM], F32, tag="inv_cur")

        cur_r = cur.rearrange("(k p) n -> p k n", p=KP)
        ref_r = ref.rearrange("(k p) n -> p k n", p=KP)

        sim_pm = sim.rearrange("(mm p) n -> p mm n", p=KP)
        for _rep in range(repeats):
            # cast-DMAs: all of ref first (every output tile reads all of
            # ref), cur slice 0 early (first m-chunks + inv_cur), rest of cur
            for n in range(NN):
                sl = slice(n * NT, (n + 1) * NT)
                nc.gpsimd.dma_start(out=ref_n[n], in_=ref_r[:, :, sl])
                if n == 0:
                    nc.gpsimd.dma_start(out=cur_fr[:, :, sl], in_=cur_r[:, :, sl])
            for n in range(1, NN):
                sl = slice(n * NT, (n + 1) * NT)
                nc.gpsimd.dma_start(out=cur_fr[:, :, sl], in_=cur_r[:, :, sl])

            def ref_chain(n):
                sq = sqp.tile([KP, NK, NT], F32R, tag="sq_r", name="sq_r")
                nc.scalar.activation(sq, ref_n[n], ACTF.Square)
                pb = pspre.tile([KP, NT], F32, tag="pre_bc", name="pb")
                for k in range(NK):
                    nc.tensor.matmul(
                        pb, ones_mat, sq[:, k, :], start=(k == 0), stop=(k == NK - 1)
                    )
                nc.scalar.activation(pb, pb, ACTF.Sqrt)
                inv128 = rowp.tile([KP, NT], F32, tag="inv128", name="inv128")
                nc.vector.reciprocal(inv128, pb)
                for k in range(NK):
                    nc.vector.tensor_mul(ref_n[n][:, k, :], ref_n[n][:, k, :], inv128)

            def cur_chain(n):
                sl = slice(n * NT, (n + 1) * NT)
                csq = sqp.tile([KP, NK, NT], F32, tag="sq_c", name="sq_c")
                if n % 2 == 0:
                    nc.vector.tensor_mul(csq, cur_fr[:, :, sl], cur_fr[:, :, sl])
                else:
                    nc.scalar.activation(csq, cur_fr[:, :, sl], ACTF.Square)
                pcol = pscol.tile([KP, MPS], F32, tag="pre_col", name="pcol")
                for j in range(MPS):
                    for k in range(NK):
                        nc.tensor.matmul(
                            pcol[:, j:j + 1],
                            csq[:, k, j * MT:(j + 1) * MT],
                            ones_col,
                            start=(k == 0),
                            stop=(k == NK - 1),
                        )
                ncur = rowp.tile([KP, MPS], F32, tag="ncur", name="ncur")
                nc.scalar.activation(ncur, pcol, ACTF.Sqrt)
                nc.vector.reciprocal(inv_cur[:, n * MPS:(n + 1) * MPS], ncur)

            def out_group(mo, msz):
                out_sb = outp.tile([KP, MO, HW], F32, tag="out", name="out_sb")
                for mi in range(msz):
                    m = mo + mi
                    for n in range(NN):
                        ps = psmm.tile([KP, NT], F32, tag="mm", name="ps")
                        for k in range(NK):
                            nc.tensor.matmul(
                                ps,
                                cur_fr[:, k, m * MT:(m + 1) * MT],
                                ref_n[n][:, k, :],
                                start=(k == 0),
                                stop=(k == NK - 1),
                            )
                        dst = out_sb[:, mi, n * NT:(n + 1) * NT]
                        if n % 2 == 0:
                            nc.scalar.mul(dst, ps, inv_cur[:, m:m + 1])
                        else:
                            nc.vector.tensor_scalar_mul(dst, ps, inv_cur[:, m:m + 1])
                nc.sync.dma_start(
                    out=sim_pm[:, mo:mo + msz, :], in_=out_sb[:, :msz, :]
                )

            # ref chains first (every output column needs all of ref);
            # cur chains 0-1 cover the first 8 m-chunks, the rest are
            # emitted after the first output groups so the first 6 MB of
            # output copies don't queue behind them on DVE/ACT.
            for n in range(NN):
                ref_chain(n)
                if n < 2:
                    cur_chain(n)

            groups = [1, 2, 3, 3] + [3] * 7 + [2]
            mo = 0
            for gi, msz in enumerate(groups):
                out_group(mo, msz)
                mo += msz
                if gi == 2:
                    for n in range(2, NN):
                        cur_chain(n)
            assert mo == NM


_NC_CACHE = {}


def _get_nc(repeats=1):
    key = ("nc", repeats)
    if key not in _NC_CACHE:
        nc = bacc.Bacc("TRN2", target_bir_lowering=False, debug=False)
        cur_d = nc.dram_tensor("cur", [C, HW], F32, kind="ExternalInput")
        ref_d = nc.dram_tensor("ref", [C, HW], F32, kind="ExternalInput")
        sim_d = nc.dram_tensor("sim", [HW, HW], F32, kind="ExternalOutput")
        with tile.TileContext(nc) as tc:
            _kernel_body(tc, cur_d.ap(), ref_d.ap(), sim_d.ap(), repeats=repeats)
        nc.compile()
        _NC_CACHE[key] = nc
    return _NC_CACHE[key]


def kernel(ref_features, cur_features, _run_kwargs=None):
    ref_np = np.ascontiguousarray(
        np.asarray(ref_features, dtype=np.float32).reshape(B, C, HW)
    )
    cur_np = np.ascontiguousarray(
        np.asarray(cur_features, dtype=np.float32).reshape(B, C, HW)
    )
    nc = _get_nc()
    in_maps = [{"cur": cur_np[b], "ref": ref_np[b]} for b in range(B)]
    res = bass_utils.run_bass_kernel_spmd(
        nc, in_maps, core_ids=list(range(B)), **(_run_kwargs or {})
    )
    out = np.stack([res.results[b]["sim"] for b in range(B)], axis=0)
    if _run_kwargs is not None:
        _NC_CACHE["last_results"] = res
    return out.reshape(B, H, W, H, W)


# revision 19
# speedup vs baseline: 23.8520x; 21.1799x over previous
"""Trainium2 Bass kernel for CorrCosine cost volumes.

Inputs (full): ref_features, cur_features [8, 256, 64, 64] f32.
out[b, hc, wc, hr, wr] = <cur_n[b, :, hc, wc], ref_n[b, :, hr, wr]>
where *_n are channel-L2-normalized features.

Sharding: data-parallel over batch B=8 across the 8 NeuronCores — each core
computes one batch's full [4096, 4096] cosine-similarity matrix:
  sim = (cur / |cur|_C).T @ (ref / |ref|_C)   with K = C = 256.

Per-core structure (Tile framework):
  - both inputs stream in 512-pixel slices [128, 2, 512] as SWDGE cast-DMAs
    (f32 -> float32r rounds in flight; f32r = 8e11m fp32 that runs the PE at
    full speed for moving dim >= 256, ~2^-12 rounding). The SP HWDGE queue
    carries only the 64 MB of output stores.
  - ref slices: square (ACT, f32r out) -> one all-ones [128,128] f32r matmul
    per K-chunk, which computes the partition-reduce AND replicates the
    sums across all partitions in one shot -> in-place PSUM sqrt (ACT) ->
    reciprocal (DVE) -> normalized in place.
  - cur stays unnormalized; inverse norms are reduced into output-row layout
    [128, 32] via N=1 matmuls and folded into the mandatory PSUM->SBUF
    copies (ACT activation scale / DVE tensor_scalar) for free.
  - main loop: 32 m-chunks x 8 n-tiles x 2 K-chunks of float32r matmuls,
    fused scaled copies alternating ScalarE/VectorE, up to 6 MB output DMAs
    (ramped group sizes [1,2,3,...] so the first store starts ~25 us in)
    on the SP queue. Model (CoreSim cost model): ~230 us/core; measured on
    HW via repeat-slope: ~160-240 us/core (DMA-roofline-bound; 72 MB of
    HBM traffic at ~360 GB/s is ~200 us).
"""

import numpy as np

import concourse.bass as bass
import concourse.mybir as mybir
import concourse.tile as tile
from concourse import bacc, bass_utils

B, C, H, W = 8, 256, 64, 64
HW = H * W           # 4096 pixels
KP = 128             # partitions per K-chunk
NK = C // KP         # 2 K-chunks
MT = 128             # output partition tile (cur pixels)
NT = 512             # output free tile (ref pixels) = one f32 PSUM bank
NM = HW // MT        # 32 m-chunks
NN = HW // NT        # 8 n-tiles
MO = 3               # m-chunks per output DMA (6 MB per dma_start)
MPS = NT // MT       # m-chunks per 512-pixel input slice = 4

F32 = mybir.dt.float32
F32R = mybir.dt.float32r
ACTF = mybir.ActivationFunctionType


def _kernel_body(tc, cur, ref, sim, repeats=1):
    nc = tc.nc
    with (
        tc.tile_pool(name="pers", bufs=1) as pers,
        tc.tile_pool(name="sqp", bufs=2) as sqp,
        tc.tile_pool(name="rowp", bufs=2) as rowp,
        tc.tile_pool(name="outp", bufs=2) as outp,
        tc.tile_pool(name="psmm", bufs=5, space=bass.MemorySpace.PSUM) as psmm,
        tc.tile_pool(name="pspre", bufs=2, space=bass.MemorySpace.PSUM) as pspre,
        tc.tile_pool(name="pscol", bufs=1, space=bass.MemorySpace.PSUM) as pscol,
    ):
        ones_col = pers.tile([KP, 1], F32, tag="ones_col")
        nc.vector.memset(ones_col, 1.0)
        # all-ones [128, 128] f32r stationary operand: ones_mat.T @ sq gives
        # the per-column sums replicated across all 128 partitions, fusing
        # the partition-reduce and the broadcast into one full-speed matmul
        ones_f32 = pers.tile([KP, KP], F32, tag="ones_f32")
        nc.vector.memset(ones_f32, 1.0)
        ones_mat = pers.tile([KP, KP], F32R, tag="ones_mat")
        nc.scalar.copy(ones_mat, ones_f32)

        cur_fr = pers.tile([KP, NK, HW], F32R, tag="cur_fr")
        ref_n = [
            pers.tile([KP, NK, NT], F32R, tag=f"ref_n{n}", name=f"ref_n{n}")
            for n in range(NN)
        ]
        inv_cur = pers.tile([KP, NM], F32, tag="inv_cur")

        cur_r = cur.rearrange("(k p) n -> p k n", p=KP)
        ref_r = ref.rearrange("(k p) n -> p k n", p=KP)

        sim_pm = sim.rearrange("(mm p) n -> p mm n", p=KP)
        for _rep in range(repeats):
            # cast-DMAs: all of ref first (every output tile reads all of
            # ref), cur slice 0 early (first m-chunks + inv_cur), rest of cur
            for n in range(NN):
                sl = slice(n * NT, (n + 1) * NT)
                nc.gpsimd.dma_start(out=ref_n[n], in_=ref_r[:, :, sl])
                if n == 0:
                    nc.gpsimd.dma_start(out=cur_fr[:, :, sl], in_=cur_r[:, :, sl])
            for n in range(1, NN):
                sl = slice(n * NT, (n + 1) * NT)
                nc.gpsimd.dma_start(out=cur_fr[:, :, sl], in_=cur_r[:, :, sl])

            def ref_chain(n):
                sq = sqp.tile([KP, NK, NT], F32R, tag="sq_r", name="sq_r")
                nc.scalar.activation(sq, ref_n[n], ACTF.Square)
                pb = pspre.tile([KP, NT], F32, tag="pre_bc", name="pb")
                for k in range(NK):
                    nc.tensor.matmul(
                        pb, ones_mat, sq[:, k, :], start=(k == 0), stop=(k == NK - 1)
                    )
                nc.scalar.activation(pb, pb, ACTF.Sqrt)
                inv128 = rowp.tile([KP, NT], F32, tag="inv128", name="inv128")
                nc.vector.reciprocal(inv128, pb)
                for k in range(NK):
                    nc.vector.tensor_mul(ref_n[n][:, k, :], ref_n[n][:, k, :], inv128)

            def cur_chain(n):
                sl = slice(n * NT, (n + 1) * NT)
                csq = sqp.tile([KP, NK, NT], F32, tag="sq_c", name="sq_c")
                if n % 2 == 0:
                    nc.vector.tensor_mul(csq, cur_fr[:, :, sl], cur_fr[:, :, sl])
                else:
                    nc.scalar.activation(csq, cur_fr[:, :, sl], ACTF.Square)
                pcol = pscol.tile([KP, MPS], F32, tag="pre_col", name="pcol")
                for j in range(MPS):
                    for k in range(NK):
                        nc.tensor.matmul(
                            pcol[:, j:j + 1],
                            csq[:, k, j * MT:(j + 1) * MT],
                            ones_col,
                            start=(k == 0),
                            stop=(k == NK - 1),
                        )
                ncur = rowp.tile([KP, MPS], F32, tag="ncur", name="ncur")
                nc.scalar.activation(ncur, pcol, ACTF.Sqrt)
                nc.vector.reciprocal(inv_cur[:, n * MPS:(n + 1) * MPS], ncur)

            def out_group(mo, msz):
                out_sb = outp.tile([KP, MO, HW], F32, tag="out", name="out_sb")
                for mi in range(msz):
                    m = mo + mi
                    for n in range(NN):
                        ps = psmm.tile([KP, NT], F32, tag="mm", name="ps")
                        for k in range(NK):
                            nc.tensor.matmul(
                                ps,
                                cur_fr[:, k, m * MT:(m + 1) * MT],
                                ref_n[n][:, k, :],
                                start=(k == 0),
                                stop=(k == NK - 1),
                            )
                        dst = out_sb[:, mi, n * NT:(n + 1) * NT]
                        if n % 2 == 0:
                            nc.scalar.mul(dst, ps, inv_cur[:, m:m + 1])
                        else:
                            nc.vector.tensor_scalar_mul(dst, ps, inv_cur[:, m:m + 1])
                nc.sync.dma_start(
                    out=sim_pm[:, mo:mo + msz, :], in_=out_sb[:, :msz, :]
                )

            # ref chains first (every output column needs all of ref);
            # cur chains 0-1 cover the first 8 m-chunks, the rest are
            # emitted after the first output groups so the first 6 MB of
            # output copies don't queue behind them on DVE/ACT.
            for n in range(NN):
                ref_chain(n)
                if n < 2:
                    cur_chain(n)

            groups = [1, 2, 3, 3] + [3] * 7 + [2]
            mo = 0
            for gi, msz in enumerate(groups):
                out_group(mo, msz)
                mo += msz
                if gi == 2:
                    for n in range(2, NN):
                        cur_chain(n)
            assert mo == NM


_NC_CACHE = {}


def _get_nc(repeats=1):
    key = ("nc", repeats)
    if key not in _NC_CACHE:
        nc = bacc.Bacc("TRN2", target_bir_lowering=False, debug=False)
        cur_d = nc.dram_tensor("cur", [C, HW], F32, kind="ExternalInput")
        ref_d = nc.dram_tensor("ref", [C, HW], F32, kind="ExternalInput")
        sim_d = nc.dram_tensor("sim", [HW, HW], F32, kind="ExternalOutput")
        with tile.TileContext(nc) as tc:
            _kernel_body(tc, cur_d.ap(), ref_d.ap(), sim_d.ap(), repeats=repeats)
        nc.compile()
        _NC_CACHE[key] = nc
    return _NC_CACHE[key]


def kernel(ref_features, cur_features, _run_kwargs=None):
    ref_np = np.ascontiguousarray(
        np.asarray(ref_features, dtype=np.float32).reshape(B, C, HW)
    )
    cur_np = np.ascontiguousarray(
        np.asarray(cur_features, dtype=np.float32).reshape(B, C, HW)
    )
    nc = _get_nc()
    in_maps = [{"cur": cur_np[b], "ref": ref_np[b]} for b in range(B)]
    res = bass_utils.run_bass_kernel_spmd(
        nc, in_maps, core_ids=list(range(B)), **(_run_kwargs or {})
    )
    out = np.stack([res.results[b]["sim"] for b in range(B)], axis=0)
    if _run_kwargs is not None:
        _NC_CACHE["last_results"] = res
    return out.reshape(B, H, W, H, W)
